# revision 20
# baseline (speedup 1.0000x reference)
"""DeepSeek-V3 token-choice top-k router on 8 Trainium2 NeuronCores.

Strategy (per core, data-parallel over tokens; 1024 tokens/core):
  - Host pre-transposes x to d-major and packs per k-chunk, so no PE
    transposes are needed: per chunk k the PE computes
    logitsT[e, t] += W_k[d, e].T @ x_k[d, t] with W as the stationary
    operand and tokens streaming (N = batch size columns).
  - Exact fp32 -grade logits via 3 fp32r passes: x split on device into
    hi + lo (exact Sterbenz split, ACT cast + DVE subtract), W split on
    host into hi + lo fp32r.  Accumulated terms: xh@Wh + xl@Wh + xh@Wl
    (dropped xl@Wl is ~2^-26 relative).
  - Tokens processed in batches [512, 256, 256]; each batch's PSUM
    logitsT ([128e, TB] x 2 expert halves) -> ACT sigmoid -> PE
    transpose back to token-major [128t, 256e] -> DVE/ACT routing
    (group top-2 sums, top-4 groups, masked top-8, one-hot gathers,
    normalize).  A batch's routing is emitted interleaved into the next
    batch's chunk loop so only the last (256-token) batch's routing is
    an exposed tail.
  - Batches 0 and 2 accumulate the big term (xh@Wh) and the ~2^-12-
    scale small terms (xl@Wh + xh@Wl) in SEPARATE PSUM banks, merged
    once at the end (ACT copy + DVE add).  This keeps the 112 small
    adds from rounding at full logit scale and was measured to remove
    the near-tie index flips vs the fp32 reference (margins down to
    2e-8 in score space survive).  Batch 1 stays single-bank to fit
    the 8 PSUM banks alongside the routing transpose tiles.
  - W DMAs ride the Activation HWDGE queue, the x stream rides the Sync
    queue (2-chunk transfers, ring prefetch).
"""

import numpy as np

N = 8192
D = 7168
E = 256
G = 8
EPG = E // G  # 32
TOPK_GROUP = 4
TOP_K = 8
SCALING = 2.5
N_CORES = 8
NPC = N // N_CORES  # 1024 tokens per core
P = 128
KC = D // P  # 56 contraction chunks
BATCHES = [512, 256, 256]
# k-chunks per DMA/split unit, per batch: batch 0 starts with two singles
# (fast pipeline fill) then pairs; batches 1/2 use 4-chunk units (their
# tokens-per-chunk is half, so the byte size matches batch 0's pairs).
UNITS_B = {0: [1, 1] + [2] * 27, 1: [4] * 14, 2: [4] * 14}
GW_UNITS = UNITS_B[0]
NGW = len(GW_UNITS)
GW_OFF = [sum(GW_UNITS[:i]) for i in range(NGW)]
# chunk index -> (gw tile index, chunk offset within tile)
GW_MAP = []
for _ti, _cnt in enumerate(GW_UNITS):
    for _j in range(_cnt):
        GW_MAP.append((_ti, _j))

_CACHE = {}


def build_program():
    import concourse.bacc as bacc
    import concourse.mybir as mybir
    from concourse import tile, masks

    nc = bacc.Bacc(
        "TRN2",
        target_bir_lowering=False,
        debug=False,
        enable_asserts=True,
        num_devices=N_CORES,
    )
    f32 = mybir.dt.float32
    f32r = mybir.dt.float32r
    i32 = mybir.dt.int32
    u32 = mybir.dt.uint32
    AF = mybir.ActivationFunctionType
    OP = mybir.AluOpType
    AX = mybir.AxisListType

    XCOLS = KC * NPC  # 57344
    x_d = nc.dram_tensor("x", [P, XCOLS], f32, kind="ExternalInput").ap()
    gw_d = nc.dram_tensor("gw", [P, KC * 4 * P], f32r, kind="ExternalInput").ap()
    bias_d = nc.dram_tensor("bias", [1, E], f32, kind="ExternalInput").ap()
    idx_d = nc.dram_tensor("idx", [NPC, TOP_K], i32, kind="ExternalOutput").ap()
    w_d = nc.dram_tensor("w", [NPC, TOP_K], f32, kind="ExternalOutput").ap()

    # column offset of batch b in the packed x layout
    xoff = []
    o = 0
    for TB in BATCHES:
        xoff.append(o)
        o += KC * TB

    with tile.TileContext(nc) as tc:
        with (
            tc.tile_pool(name="const", bufs=1) as const_pool,
            tc.tile_pool(name="gw", bufs=NGW) as gw_pool,
            tc.tile_pool(name="x", bufs=6) as x_pool,
            tc.tile_pool(name="xs", bufs=4) as xs_pool,
            tc.tile_pool(name="plogA", bufs=4, space="PSUM") as plogA_pool,
            tc.tile_pool(name="plogB", bufs=2, space="PSUM") as plogB_pool,
            tc.tile_pool(name="psc", bufs=2, space="PSUM") as psc_pool,
            tc.tile_pool(name="st", bufs=4) as st_pool,
            tc.tile_pool(name="work", bufs=2) as work_pool,
            tc.tile_pool(name="outs", bufs=4) as out_pool,
        ):
            # ---- W rides the ACT HWDGE queue: a few units upfront, the rest
            # ---- dripped inside batch 0's chunk loop (prefetch offset +3)
            # ---- so the 14.7MB doesn't monopolize the queue or the engine.
            gw_tiles = []

            def issue_gw(kk, eng=None):
                c0, cn = GW_OFF[kk], GW_UNITS[kk]
                g = gw_pool.tile([P, 2 * 4 * P], f32r, tag="gwt", name=f"gw{kk}")
                (eng or nc.scalar).dma_start(
                    g[:, : cn * 4 * P],
                    gw_d[:, c0 * 4 * P : (c0 + cn) * 4 * P],
                )
                gw_tiles.append(g)

            # first x unit + first gw unit ride the sync queue ahead of the
            # ACT queue's table-load stall; bias follows (needed much later)
            x_first = x_pool.tile([P, 2 * 512], f32, tag="xt", name="x0_0")
            nc.sync.dma_start(x_first[:, : UNITS_B[0][0] * 512], x_d[:, : UNITS_B[0][0] * 512])
            issue_gw(0, eng=nc.sync)
            bias_sb = const_pool.tile([1, E], f32, name="biassb")
            nc.sync.dma_start(bias_sb[:], bias_d[:])
            for kk in range(1, 4):
                issue_gw(kk)

            # ---- constants ----
            ident = const_pool.tile([P, P], f32)
            masks.make_identity(nc, ident[:])
            iota_i = const_pool.tile([P, E], i32)
            nc.gpsimd.iota(iota_i[:], pattern=[[1, E]], base=0, channel_multiplier=0)
            iota_f = const_pool.tile([P, E], f32)
            nc.vector.tensor_copy(iota_f[:], iota_i[:])
            bias_rep = const_pool.tile([P, E], f32)
            nc.gpsimd.partition_broadcast(bias_rep[:], bias_sb[0:1, :])

            def routing_tile(sfcP, gt0):
                """Route one 128-token tile; sfcP = [128t, 256e] scores in PSUM."""
                sfc = work_pool.tile([P, E], f32, tag="sfc", bufs=3)
                nc.vector.tensor_tensor(sfc[:], sfcP[:], bias_rep[:], op=OP.add)

                # per-group top-8 (need top-2 of each group of 32)
                gtops = work_pool.tile([P, G * 8], f32, tag="gtops")
                for g in range(G):
                    nc.vector.max(
                        gtops[:, g * 8 : (g + 1) * 8],
                        sfc[:, g * EPG : (g + 1) * EPG],
                    )
                gv = gtops[:].rearrange("p (g k) -> p g k", g=G)
                gs = work_pool.tile([P, G], f32, tag="gs")
                nc.vector.tensor_tensor(gs[:], gv[:, :, 0], gv[:, :, 1], op=OP.add)

                # top-4 groups -> mask
                gtop8 = work_pool.tile([P, 8], f32, tag="gtop8")
                nc.vector.max(gtop8[:], gs[:])
                gmask = work_pool.tile([P, G], f32, tag="gmask")
                nc.vector.tensor_scalar(
                    gmask[:], gs[:], gtop8[:, TOPK_GROUP - 1 : TOPK_GROUP], None,
                    op0=OP.is_ge,
                )

                # masked scores
                tmp = work_pool.tile([P, E], f32, tag="tmp")
                for g in range(G):
                    nc.vector.tensor_scalar(
                        tmp[:, g * EPG : (g + 1) * EPG],
                        sfc[:, g * EPG : (g + 1) * EPG],
                        gmask[:, g : g + 1],
                        None,
                        op0=OP.mult,
                    )

                # top-8 values + indices
                vals = work_pool.tile([P, TOP_K], f32, tag="vals")
                nc.vector.max(vals[:], tmp[:])
                idxu = work_pool.tile([P, TOP_K], u32, tag="idxu")
                nc.vector.max_index(idxu[:], vals[:], tmp[:])
                idxf = work_pool.tile([P, TOP_K], f32, tag="idxf")
                nc.vector.tensor_copy(idxf[:], idxu[:])

                # gather raw sigmoid scores at the selected indices
                # (sfcP holds the raw transposed-back sigmoid scores)
                w8 = out_pool.tile([P, TOP_K], f32, tag="w8")
                scratch = work_pool.tile([P, E], f32, tag="scratch")
                for j in range(TOP_K):
                    nc.vector.scalar_tensor_tensor(
                        scratch[:],
                        iota_f[:],
                        idxf[:, j : j + 1],
                        sfcP[:],
                        op0=OP.is_equal,
                        op1=OP.mult,
                        accum_out=w8[:, j : j + 1],
                    )

                # normalize + scale
                wsum = work_pool.tile([P, 1], f32, tag="wsum")
                nc.vector.reduce_sum(wsum[:], w8[:], axis=AX.X)
                # reference adds 1e-20 before the reciprocal; wsum >= ~0.5 so
                # the fp32 add is bitwise a no-op and is elided here
                wrec = work_pool.tile([P, 1], f32, tag="wrec")
                nc.vector.reciprocal(wrec[:], wsum[:])
                w_out = out_pool.tile([P, TOP_K], f32, tag="wout")
                nc.vector.tensor_scalar(
                    w_out[:], w8[:], wrec[:, 0:1], float(SCALING),
                    op0=OP.mult, op1=OP.mult,
                )
                idx_out = out_pool.tile([P, TOP_K], i32, tag="idxout")
                nc.vector.tensor_copy(idx_out[:], idxu[:])

                nc.sync.dma_start(idx_d[gt0 : gt0 + P, :], idx_out[:])
                nc.sync.dma_start(w_d[gt0 : gt0 + P, :], w_out[:])

            def epilogue_steps(b, pA, pB):
                """Closures: psum-merge + sigmoid head, then one routing/tile
                (each routing step transposes its own token tile first)."""
                TB = BATCHES[b]
                t0 = sum(BATCHES[:b])
                NT = TB // P
                state = {}

                def head():
                    ss = []
                    for h in range(2):
                        if pB is not None:
                            bsb = st_pool.tile([P, TB], f32, tag="bsb", bufs=2,
                                               name=f"bsb{b}{h}")
                            nc.scalar.copy(bsb[:], pB[h][:])
                            lsum = st_pool.tile([P, TB], f32, tag="lsum", bufs=2,
                                                name=f"lsum{b}{h}")
                            nc.vector.tensor_tensor(
                                lsum[:], pA[h][:], bsb[:], op=OP.add
                            )
                            src = lsum
                        else:
                            src = pA[h]
                        s = st_pool.tile([P, TB], f32, tag="sct", name=f"sct{b}{h}")
                        nc.scalar.activation(s[:], src[:], AF.Sigmoid)
                        ss.append(s)
                    state["ss"] = ss

                steps = [head]
                for tt in range(NT):
                    def rt(tt=tt):
                        s0, s1 = state["ss"]
                        sp = psc_pool.tile([P, E], f32, tag="psc")
                        nc.tensor.matmul(
                            sp[:, 0:P], s0[:, tt * P : (tt + 1) * P], ident[:],
                            is_transpose=True,
                        )
                        nc.tensor.matmul(
                            sp[:, P:E], s1[:, tt * P : (tt + 1) * P], ident[:],
                            is_transpose=True,
                        )
                        routing_tile(sp, t0 + tt * P)
                    steps.append(rt)
                return steps

            pending = []  # epilogue closures of the previous batch

            for b, TB in enumerate(BATCHES):
                UNITS = UNITS_B[b]
                NU = len(UNITS)
                UOFF = [sum(UNITS[:i]) for i in range(NU)]
                UC2 = 1024  # pool slot size (max unit columns, all batches)
                two_bank = b in (0, 2)
                pA = [
                    plogA_pool.tile([P, TB], f32, tag="plogA", name=f"pA{b}h{h}")
                    for h in range(2)
                ]
                pB = (
                    [
                        plogB_pool.tile([P, TB], f32, tag="plogB", name=f"pB{b}h{h}")
                        for h in range(2)
                    ]
                    if two_bank
                    else None
                )
                pS = pB if two_bank else pA  # where small terms accumulate
                for kk in range(NU):
                    c0, cn = UOFF[kk], UNITS[kk]
                    UC = cn * TB
                    if b == 0 and kk == 0:
                        x_t = x_first
                    else:
                        x_t = x_pool.tile([P, UC2], f32, tag="xt", name=f"x{b}_{kk}")
                        nc.sync.dma_start(
                            x_t[:, :UC],
                            x_d[:, xoff[b] + c0 * TB : xoff[b] + (c0 + cn) * TB],
                        )
                    xh = xs_pool.tile([P, UC2], f32r, tag="xh")
                    nc.scalar.copy(xh[:, :UC], x_t[:, :UC])
                    xl = xs_pool.tile([P, UC2], f32r, tag="xl")
                    nc.vector.scalar_tensor_tensor(
                        xl[:, :UC], x_t[:, :UC], 0.0, xh[:, :UC].bitcast(f32),
                        op0=OP.add, op1=OP.subtract,
                    )
                    if b == 0 and kk + 4 < NGW:
                        issue_gw(kk + 4)
                    for j in range(cn):
                        k = c0 + j
                        ti, wj = GW_MAP[k]
                        g = gw_tiles[ti]
                        wb = wj * 4 * P
                        xs = slice(j * TB, (j + 1) * TB)
                        first = k == 0
                        last = k == KC - 1
                        # big terms: xh @ Wh (both halves)
                        nc.tensor.matmul(
                            pA[0][:], g[:, wb : wb + P], xh[:, xs],
                            start=first, stop=(last and two_bank),
                        )
                        nc.tensor.matmul(
                            pA[1][:], g[:, wb + P : wb + 2 * P], xh[:, xs],
                            start=first, stop=(last and two_bank),
                        )
                        # small terms: xl @ Wh + xh @ Wl
                        nc.tensor.matmul(
                            pS[0][:], g[:, wb : wb + P], xl[:, xs],
                            start=(first and two_bank), stop=False,
                        )
                        nc.tensor.matmul(
                            pS[1][:], g[:, wb + P : wb + 2 * P], xl[:, xs],
                            start=(first and two_bank), stop=False,
                        )
                        nc.tensor.matmul(
                            pS[0][:], g[:, wb + 2 * P : wb + 3 * P], xh[:, xs],
                            start=False, stop=last,
                        )
                        nc.tensor.matmul(
                            pS[1][:], g[:, wb + 3 * P : wb + 4 * P], xh[:, xs],
                            start=False, stop=last,
                        )
                    # drip-feed pending epilogue steps between chunks; steps
                    # carry across batch boundaries so earlier batches' routing
                    # spreads over the whole remaining stream instead of
                    # oversubscribing the next batch's DVE window
                    if pending and kk >= 1 and kk % 4 == 1:
                        pending.pop(0)()
                pending.extend(epilogue_steps(b, pA, pB))

            while pending:
                pending.pop(0)()

    nc.compile()
    return nc


def _get_nc(**kw):
    key = tuple(sorted(kw.items()))
    if key not in _CACHE:
        _CACHE[key] = build_program(**kw)
    return _CACHE[key]


def _fp32r_round(a):
    # round-to-nearest fp32 -> fp32r (12-bit mantissa), bit-exact with HW cast
    bits = np.ascontiguousarray(a).view(np.uint32)
    keep = np.uint32(0xFFFFF000)
    rounded = (bits + np.uint32(0x800)) & keep  # round-half-up into kept bits
    # correct round-to-nearest-even on the halfway case
    half = (bits & np.uint32(0xFFF)) == np.uint32(0x800)
    even = ((bits >> np.uint32(12)) & np.uint32(1)) == 0
    rounded = np.where(half & even, bits & keep, rounded)
    return rounded.view(np.float32).reshape(a.shape)


def _pack_x_core(xc):
    """[1024, 7168] fp32 -> [128, 56*1024] d-major, batch-then-chunk packed."""
    parts = []
    t0 = 0
    for TB in BATCHES:
        xb = xc[t0 : t0 + TB]  # [TB, D]
        xb = np.ascontiguousarray(
            xb.reshape(TB, KC, P).transpose(2, 1, 0)
        ).reshape(P, KC * TB)
        parts.append(xb)
        t0 += TB
    return np.ascontiguousarray(np.concatenate(parts, axis=1))


def _pack_w(gate_w):
    """[256, 7168] fp32 -> [128, 56*512] per-chunk [Wh_h0|Wh_h1|Wl_h0|Wl_h1]."""
    gwt = np.ascontiguousarray(gate_w.T)  # [D, E]
    wh = _fp32r_round(gwt)
    wl = gwt - wh  # exact; fits in fp32r
    blocks = []
    for k in range(KC):
        bh = wh[k * P : (k + 1) * P]
        bl = wl[k * P : (k + 1) * P]
        blocks.append(
            np.concatenate(
                [bh[:, :P], bh[:, P:], bl[:, :P], bl[:, P:]], axis=1
            )
        )
    return np.ascontiguousarray(np.concatenate(blocks, axis=1))


def _host_pack(x, gate_w, bias):
    x = np.ascontiguousarray(np.asarray(x, dtype=np.float32))
    gate_w = np.ascontiguousarray(np.asarray(gate_w, dtype=np.float32))
    bias = np.ascontiguousarray(np.asarray(bias, dtype=np.float32))
    gw = _pack_w(gate_w)
    bias2d = bias.reshape(1, E)
    xs = [_pack_x_core(x[c * NPC : (c + 1) * NPC]) for c in range(N_CORES)]
    return xs, gw, bias2d


def _run(x, gate_w, bias, trace=False, **build_kw):
    from concourse.bass_utils import run_bass_kernel_spmd

    nc = _get_nc(**build_kw)
    xs, gw, bias2d = _host_pack(x, gate_w, bias)
    in_maps = [{"x": xs[c], "gw": gw, "bias": bias2d} for c in range(N_CORES)]
    res = run_bass_kernel_spmd(nc, in_maps, core_ids=list(range(N_CORES)), trace=trace)
    idx = np.concatenate([res.results[c]["idx"] for c in range(N_CORES)], axis=0)
    w = np.concatenate([res.results[c]["w"] for c in range(N_CORES)], axis=0)
    return (idx.astype(np.int32), w.astype(np.float32)), res


def kernel(x, gate_w, bias):
    (idx, w), _ = _run(x, gate_w, bias)
    return idx, w
